# revision 1
# baseline (speedup 1.0000x reference)
"""Trainium2 Bass kernel for nn_CanonicalMicrocircuit (gnn_message_passing).

Math note: the reference module starts from all-zero recurrent state and only
returns `all_out * (1 - g)`, so every einsum against the zero state vanishes,
the inhibitory population and the inter-column lateral tensor are dead code,
and only layer 0 of the excitatory update survives:

    x0_c  = relu((1-exp(-1/tau_c)) * (blat_e[c,0] + bfb_e[c,0]) - thr_c)
    x0_c /= (||x0_c|| + 1e-8)
    out_c = relu(Wexc[c,0] @ x0_c + bexc[c,0])            # [H] per column
    h     = sum_c Wg1[:, cH:(c+1)H] @ out_c + bg1         # [H]
    r     = relu(h)
    g_c   = sigmoid(Wg2[cH:(c+1)H, :] @ r + bg2[cH:(c+1)H])
    final_c = out_c * (1 - g_c)                           # concat -> [C*H]

Sharding: one column per NeuronCore (C == 8 == n_cores).  Each core holds its
column's Wexc slice plus the matching column-block of Wg1 and row-block of
Wg2.  The only communication is one 4 KB AllGather of the per-core Wg1
partial products, summed locally on every core.

Engine plan (from profiling): the runtime inserts a collective-init barrier
on the CC stream at kernel entry (~46 us here) that also gates the Tensor
queue, and the ncfw AllGather costs ~39 us after trigger.  So stages A and B
run on DVE+GpSimd (scalar_tensor_tensor with accum_out = per-row dot
products against partition-broadcast vectors), pipelined behind the weight
DMAs and finishing before the barrier clears; the AllGather triggers as
early as its input exists; stage C (post-AllGather) is split between the PE
(rows 0-511, host-pre-transposed shard) and DVE (rows 512-1023, natural
shard) to shorten the tail.
"""

import numpy as np

import concourse.bass as bass
import concourse.bacc as bacc
import concourse.mybir as mybir
import concourse.tile as tile
from concourse.bass_utils import run_bass_kernel_spmd

C = 8
F = 512
L = 4
H = 1024
HI = 256
NCORES = 8
P = 128
KT = H // P  # 8 row/k tiles per 1024 dim
FP = mybir.dt.float32
TOP = 384  # stage-C rows on the PE
BOT = H - TOP
KB = BOT // P  # 5 DVE row-tiles in stage C

_CACHE = {}


def _build_nc():
    nc = bacc.Bacc(
        "TRN2",
        target_bir_lowering=False,
        debug=False,
        enable_asserts=False,
        num_devices=NCORES,
    )

    w1 = nc.dram_tensor("w1", [H, H], FP, kind="ExternalInput")  # Wexc[c,0] natural
    w2 = nc.dram_tensor("w2", [H, H], FP, kind="ExternalInput")  # Wg1[:,blk] natural
    w3t = nc.dram_tensor("w3t", [H, TOP], FP, kind="ExternalInput")  # top.T
    w3n = nc.dram_tensor("w3n", [BOT, H], FP, kind="ExternalInput")  # bottom nat
    vecs = nc.dram_tensor("vecs", [6, H], FP, kind="ExternalInput")
    eye = nc.dram_tensor("eye", [P, P], FP, kind="ExternalInput")
    # rows (rho = p-major storage permutation, see make_in_maps):
    # 0=blat, 1=bfb, 2=bexc[rho], 3=bg1[rho], 4=[bg2p[:512], bg2p-bot-col], 5=[tau, thr]
    fin = nc.dram_tensor("final", [1, H], FP, kind="ExternalOutput")

    AF = mybir.ActivationFunctionType
    ALU = mybir.AluOpType

    with tile.TileContext(nc) as tc:
        with (
            tc.tile_pool(name="sb", bufs=1) as sb,
            tc.tile_pool(name="jk", bufs=2) as jk,
            tc.tile_pool(name="ps_row", bufs=3, space="PSUM") as ps_row,
            tc.tile_pool(name="ps_tp", bufs=1, space="PSUM") as ps_tp,
            tc.tile_pool(name="dram", bufs=1, space="DRAM") as dram,
        ):
            # ---- weight loads: SP hwdge ring, FIFO in program order ----
            # W1 and W2 as 2x 2MB chunks (4 row-tiles each) for pipelining.
            def load_nat_pairs(name, dram_t):
                tiles = []
                for a in range(KT // 4):
                    t = sb.tile([P, 4, H], FP, tag=f"{name}{a}")
                    src = dram_t.ap()[4 * a * P : 4 * (a + 1) * P, :].rearrange(
                        "(t p) i -> p t i", p=P
                    )
                    nc.sync.dma_start(t[:], src)
                    tiles.append(t)
                return tiles  # tiles[a][:, b, :] is row-tile 4a+b

            w1_t = load_nat_pairs("w1", w1)
            w2_t = load_nat_pairs("w2", w2)
            w3t_t = sb.tile([P, KT, TOP], FP, tag="w3t")
            nc.sync.dma_start(w3t_t[:], w3t.ap().rearrange("(k p) i -> p k i", p=P))
            w3n_t = sb.tile([P, KB, H], FP, tag="w3n")
            nc.sync.dma_start(w3n_t[:], w3n.ap().rearrange("(t p) i -> p t i", p=P))

            # ---- small loads on the ACT hwdge ring ----
            vt = sb.tile([1, 6 * H], FP, tag="vecs")
            nc.scalar.dma_start(
                vt[:], vecs.ap().rearrange("a b -> (a b)").rearrange("(x n) -> x n", x=1)
            )
            bexc_col = sb.tile([P, KT], FP, tag="bexc_col")
            nc.scalar.dma_start(
                bexc_col[:], vecs.ap()[2].rearrange("(p t) -> p t", p=P)
            )
            bg2_bot = sb.tile([P, KB], FP, tag="bg2_bot")
            nc.scalar.dma_start(
                bg2_bot[:], vecs.ap()[4][TOP:H].rearrange("(p t) -> p t", p=P)
            )

            # ---- constants for the PE (post-collective stages) ----
            eye_t = sb.tile([P, P], FP, tag="eye")
            nc.scalar.dma_start(eye_t[:], eye.ap())
            ones_8 = sb.tile([KT, 1], FP, tag="ones_8")
            one_11 = sb.tile([1, 1], FP, tag="one_11")
            nc.vector.memset(ones_8[:], 1.0)
            nc.vector.memset(one_11[:], 1.0)

            # ---- x0 in row form on partition 0 ----
            rt = sb.tile([1, 1], FP, tag="rt")
            nc.vector.reciprocal(rt[:], vt[0:1, 5 * H : 5 * H + 1])
            ea = sb.tile([1, 1], FP, tag="ea")
            nc.scalar.activation(ea[:], rt[:], AF.Exp, scale=-1.0)  # exp(-1/tau)
            oma = sb.tile([1, 1], FP, tag="oma")
            nc.scalar.activation(oma[:], ea[:], AF.Copy, scale=-1.0, bias=1.0)
            nthr = sb.tile([1, 1], FP, tag="nthr")
            nc.scalar.activation(nthr[:], vt[0:1, 5 * H + 1 : 5 * H + 2], AF.Copy, scale=-1.0)

            xr = sb.tile([1, H], FP, tag="xr")
            nc.vector.tensor_add(xr[:], vt[0:1, 0:H], vt[0:1, H : 2 * H])
            nc.vector.tensor_scalar(
                xr[:], xr[:], oma[:], nthr[:], op0=ALU.mult, op1=ALU.add
            )
            nc.vector.tensor_scalar_max(xr[:], xr[:], 0.0)
            ssq = sb.tile([1, 1], FP, tag="ssq")
            sqj = jk.tile([1, H], FP, tag="sqj")
            nc.vector.scalar_tensor_tensor(
                sqj[:], xr[:], 1.0, xr[:], op0=ALU.mult, op1=ALU.mult,
                accum_out=ssq[:],
            )
            nrm = sb.tile([1, 1], FP, tag="nrm")
            nc.scalar.activation(nrm[:], ssq[:], AF.Sqrt)
            nc.scalar.activation(nrm[:], nrm[:], AF.Copy, bias=1e-8)
            inv = sb.tile([1, 1], FP, tag="inv")
            nc.vector.reciprocal(inv[:], nrm[:])
            nc.vector.tensor_scalar_mul(xr[:], xr[:], inv[:])

            xb = sb.tile([P, H], FP, tag="xb")
            nc.gpsimd.partition_broadcast(xb[:], xr[0:1, :])

            # ---- fused row-dot matvec: acc[p, t] = sum_j W[t*128+p, j]*v[j]
            def matvec_nat(tiles, vb, acc):
                for t in range(KT):
                    w_ap = tiles[t // 4][:, t % 4, :]
                    junk = jk.tile([P, H], FP, tag="jv")
                    nc.vector.scalar_tensor_tensor(
                        junk[:], w_ap, 1.0, vb[:], op0=ALU.mult, op1=ALU.mult,
                        accum_out=acc[:, t : t + 1],
                    )

            # Stage A: out_c = relu(W1 @ x0 + bexc)
            outa = sb.tile([P, KT], FP, tag="outa")
            matvec_nat(w1_t, xb, outa)
            nc.vector.tensor_add(outa[:], outa[:], bexc_col[:])
            nc.vector.tensor_scalar_max(outa[:], outa[:], 0.0)
            outa_row = sb.tile([1, H], FP, tag="outa_row")
            nc.scalar.dma_start(outa_row[:], outa[:])
            xb2 = sb.tile([P, H], FP, tag="xb2")
            nc.gpsimd.partition_broadcast(xb2[:], outa_row[0:1, :])

            # Stage B: hp = W2 @ out_c + bg1/8 (so the gathered sum includes bg1)
            bg1_col = sb.tile([P, KT], FP, tag="bg1_col")
            nc.scalar.dma_start(
                bg1_col[:], vecs.ap()[3].rearrange("(p t) -> p t", p=P)
            )
            hp = sb.tile([P, KT], FP, tag="hp")
            matvec_nat(w2_t, xb2, hp)
            nc.vector.scalar_tensor_tensor(
                hp[:], bg1_col[:], 0.125, hp[:], op0=ALU.mult, op1=ALU.add
            )

            # AllGather the 4KB partials, triggered as soon as hp lands
            cc_in = dram.tile([1, H], FP, tag="cc_in")
            cc_out = dram.tile([NCORES, H], FP, tag="cc_out")
            nc.scalar.dma_start(cc_in[:], hp[:])
            nc.gpsimd.collective_compute(
                "AllGather",
                ALU.bypass,
                replica_groups=[list(range(NCORES))],
                ins=[cc_in[:]],
                outs=[cc_out[:]],
            )
            # r = relu(sum_c partials) straight into PE-ready column form:
            # per k-tile, lhsT = agt slice [8, 128] summed by a ones rhs.
            agt_a = sb.tile([NCORES, H // 2], FP, tag="agt_a")
            nc.scalar.dma_start(agt_a[:], cc_out[:, 0 : H // 2])
            agt_b = sb.tile([NCORES, H // 2], FP, tag="agt_b")
            nc.sync.dma_start(agt_b[:], cc_out[:, H // 2 : H])
            psRc = ps_tp.tile([P, KT], FP, tag="tp")
            for t in range(KT):
                half = agt_a if t < 4 else agt_b
                nc.tensor.matmul(
                    psRc[:, t : t + 1],
                    half[:, (t % 4) * P : (t % 4 + 1) * P],
                    ones_8[:],
                    start=True,
                    stop=True,
                )
            r_col = sb.tile([P, KT], FP, tag="r_col")
            nc.scalar.activation(r_col[:], psRc[:], AF.Relu)

            # Stage C rows 0-511 on the PE: s = sigmoid(-(W3 @ r + bg2))

            s_row = sb.tile([1, H], FP, tag="s_row")
            ps = ps_row.tile([1, TOP], FP, tag="row")
            for k in range(KT):
                nc.tensor.matmul(
                    ps[:],
                    r_col[:, k : k + 1],
                    w3t_t[:, k, :],
                    start=(k == 0),
                    stop=False,
                )
            nc.tensor.matmul(
                ps[:], one_11[:], vt[0:1, 4 * H : 4 * H + TOP], start=False, stop=True
            )
            nc.scalar.activation(s_row[0:1, 0:TOP], ps[:], AF.Sigmoid, scale=-1.0)

            # Stage C rows 512-1023 on DVE/GpSimd
            rrow2 = sb.tile([1, H], FP, tag="rrow2")
            nc.scalar.dma_start(rrow2[:], r_col[:])
            xb3 = sb.tile([P, H], FP, tag="xb3")
            nc.gpsimd.partition_broadcast(xb3[:], rrow2[0:1, :])
            zb = sb.tile([P, KB], FP, tag="zb")
            for t in range(KB):
                junk = jk.tile([P, H], FP, tag="jv")
                nc.vector.scalar_tensor_tensor(
                    junk[:], w3n_t[:, t, :], 1.0, xb3[:], op0=ALU.mult, op1=ALU.mult,
                    accum_out=zb[:, t : t + 1],
                )
            nc.vector.tensor_add(zb[:], zb[:], bg2_bot[:])
            # row-ize zb on the PE (identity transpose), sigmoid from PSUM
            psZ = ps_row.tile([1, BOT], FP, tag="row")
            for t in range(KB):
                nc.tensor.matmul(
                    psZ[0:1, t * P : (t + 1) * P],
                    zb[:, t : t + 1],
                    eye_t[:],
                    start=True,
                    stop=True,
                )
            nc.scalar.activation(s_row[0:1, TOP:H], psZ[:], AF.Sigmoid, scale=-1.0)

            # final = out_c * s  (rho-ordered row; host un-permutes)
            fin_a = sb.tile([1, TOP], FP, tag="fin_a")
            nc.vector.tensor_mul(fin_a[:], outa_row[0:1, 0:TOP], s_row[0:1, 0:TOP])
            nc.sync.dma_start(fin.ap()[0:1, 0:TOP], fin_a[:])
            fin_b = sb.tile([1, BOT], FP, tag="fin_b")
            nc.vector.tensor_mul(fin_b[:], outa_row[0:1, TOP:H], s_row[0:1, TOP:H])
            nc.sync.dma_start(fin.ap()[0:1, TOP:H], fin_b[:])

    nc.compile()
    return nc


def get_nc():
    if "nc" not in _CACHE:
        _CACHE["nc"] = _build_nc()
    return _CACHE["nc"]


def make_in_maps(inputs):
    """Slice the full inputs into 8 per-core input dicts (layout prep only).

    RHO is the p-major storage permutation: the device keeps the exchanged
    1024-vectors in storage order s with natural index rho[s] = (s%8)*128 +
    s//8, which makes every on-device transpose DMA contiguous.  The
    contractions are order-invariant, so we permute the matching weight
    columns / bias entries here and un-permute the final output on the host.
    """
    Wexc = np.asarray(inputs["Wexc"], dtype=np.float32)
    Wg1 = np.asarray(inputs["Wg1"], dtype=np.float32)
    Wg2 = np.asarray(inputs["Wg2"], dtype=np.float32)
    blat = np.asarray(inputs["blat_e"], dtype=np.float32)
    bfb = np.asarray(inputs["bfb_e"], dtype=np.float32)
    bexc = np.asarray(inputs["bexc"], dtype=np.float32)
    bg1 = np.asarray(inputs["bg1"], dtype=np.float32)
    bg2 = np.asarray(inputs["bg2"], dtype=np.float32)
    tau = np.asarray(inputs["tau_exc"], dtype=np.float32)
    thr = np.asarray(inputs["threshold"], dtype=np.float32)

    s_idx = np.arange(H)
    rho = (s_idx % KT) * P + s_idx // KT  # storage -> natural
    eye = np.eye(P, dtype=np.float32)

    in_maps = []
    for c in range(NCORES):
        sl = slice(c * H, (c + 1) * H)
        srow = np.zeros((H,), np.float32)
        srow[0], srow[1] = tau[c], thr[c]
        bg2p = bg2[sl][rho]
        # bottom col-form bias: row4[TOP + p*KB + t] = bg2p[TOP + t*128 + p]
        bg2_bot = bg2p[TOP:].reshape(KB, P).T.reshape(-1)
        row4 = np.concatenate([bg2p[:TOP], bg2_bot])
        vecs = np.stack([blat[c, 0], bfb[c, 0], bexc[c, 0][rho], bg1[rho], row4, srow])
        w3pp = Wg2[sl][np.ix_(rho, rho)]
        in_maps.append(
            {
                "w1": np.ascontiguousarray(Wexc[c, 0]),
                "w2": np.ascontiguousarray(Wg1[:, sl][:, rho]),
                "w3t": np.ascontiguousarray(w3pp[0:TOP, :].T),
                "w3n": np.ascontiguousarray(w3pp[TOP:, :][:, rho]),
                "vecs": np.ascontiguousarray(vecs),
                "eye": eye,
            }
        )
    return in_maps


def kernel(**inputs):
    nc = get_nc()
    in_maps = make_in_maps(inputs)
    res = run_bass_kernel_spmd(nc, in_maps, core_ids=list(range(NCORES)))
    _CACHE["last_result"] = res
    chunks = []
    for c in range(NCORES):
        st = res.results[c]["final"].reshape(P, KT)  # storage s = p*KT + t
        chunks.append(np.ascontiguousarray(st.T).reshape(-1))  # natural t*P+p
    return np.concatenate(chunks).astype(np.float32)



# revision 15
# speedup vs baseline: 1.0119x; 1.0119x over previous
"""Trainium2 Bass kernel for nn_CanonicalMicrocircuit (gnn_message_passing).

Math note: the reference module starts from all-zero recurrent state and only
returns `all_out * (1 - g)`, so every einsum against the zero state vanishes,
the inhibitory population and the inter-column lateral tensor are dead code,
and only layer 0 of the excitatory update survives:

    x0_c  = relu((1-exp(-1/tau_c)) * (blat_e[c,0] + bfb_e[c,0]) - thr_c)
    x0_c /= (||x0_c|| + 1e-8)
    out_c = relu(Wexc[c,0] @ x0_c + bexc[c,0])            # [H] per column
    h     = sum_c Wg1[:, cH:(c+1)H] @ out_c + bg1         # [H]
    r     = relu(h)
    g_c   = sigmoid(Wg2[cH:(c+1)H, :] @ r + bg2[cH:(c+1)H])
    final_c = out_c * (1 - g_c)                           # concat -> [C*H]

Sharding: one column per NeuronCore (C == 8 == n_cores).  Each core holds its
column's Wexc slice plus the matching column-block of Wg1 and row-block of
Wg2.  The only communication is one 4 KB AllGather of the per-core Wg1
partial products, summed locally on every core.

v6 (vs the 112 us baseline):
  * Weights travel as bf16 (host-cast): 6.3 MB instead of 12.6 MB of HBM
    traffic.
  * All compute on DVE/Scalar/GpSimd (row-dot matvecs with accum_out against
    partition-broadcast vectors).  The PE is unused, which removes the
    baseline's ~15 us serialized tiny-matmul tail; the gathered partials are
    pulled back from the CC output as column tiles and summed with plain
    vector adds instead of PE transposes.
  * w1 streams on the ACT hwdge ring (behind the small vecs loads) while
    w2/w3 stream on the SP ring, so stage A's weights are not queued behind
    stage B/C's.
  * Activation tables (exp/sqrt/sigmoid) are pre-warmed so their 1.3 us
    ACT_TABLE_LOADs stay off the critical path.
"""

import numpy as np
import ml_dtypes

import concourse.bass as bass
import concourse.bacc as bacc
import concourse.mybir as mybir
import concourse.tile as tile
from concourse.bass_utils import run_bass_kernel_spmd

C = 8
F = 512
L = 4
H = 1024
NCORES = 8
P = 128
KT = H // P  # 8 row/k tiles per 1024 dim
FP = mybir.dt.float32
BF = mybir.dt.bfloat16

_CACHE = {}


def _build_nc():
    nc = bacc.Bacc(
        "TRN2",
        target_bir_lowering=False,
        debug=False,
        enable_asserts=False,
        num_devices=NCORES,
    )

    w1 = nc.dram_tensor("w1", [H, H], BF, kind="ExternalInput")  # Wexc[c,0] natural
    w2 = nc.dram_tensor("w2", [H, H], BF, kind="ExternalInput")  # Wg1[:,blk][:,rho]
    w3 = nc.dram_tensor("w3", [H, H], BF, kind="ExternalInput")  # Wg2[blk][:,rho]
    vecs = nc.dram_tensor("vecs", [6, H], FP, kind="ExternalInput")
    # rows (rho = p-major storage permutation, see make_in_maps):
    # 0=blat, 1=bfb, 2=bexc[rho], 3=bg1[rho], 4=bg2[rho], 5=[tau, thr]
    fin = nc.dram_tensor("final", [P, KT], FP, kind="ExternalOutput")

    AF = mybir.ActivationFunctionType
    ALU = mybir.AluOpType

    with tile.TileContext(nc) as tc:
        with (
            tc.tile_pool(name="sb", bufs=1) as sb,
            tc.tile_pool(name="jk", bufs=2) as jk,
            tc.tile_pool(name="dram", bufs=1, space="DRAM") as dram,
        ):
            # ---- stage B/C weights on the SP hwdge ring ----
            def load_nat_pairs(name, dram_t, engine):
                tiles = []
                for a in range(KT // 4):
                    t = sb.tile([P, 4, H], BF, tag=f"{name}{a}")
                    src = dram_t.ap()[4 * a * P : 4 * (a + 1) * P, :].rearrange(
                        "(t p) i -> p t i", p=P
                    )
                    engine.dma_start(t[:], src)
                    tiles.append(t)
                return tiles  # tiles[a][:, b, :] is row-tile 4a+b

            w2_t = load_nat_pairs("w2", w2, nc.sync)
            w3_t = load_nat_pairs("w3", w3, nc.sync)

            # ---- small loads + stage-A weights on the ACT hwdge ring ----
            vt = sb.tile([1, 6 * H], FP, tag="vecs")
            nc.scalar.dma_start(
                vt[:], vecs.ap().rearrange("a b -> (a b)").rearrange("(x n) -> x n", x=1)
            )
            bexc_col = sb.tile([P, KT], FP, tag="bexc_col")
            nc.scalar.dma_start(
                bexc_col[:], vecs.ap()[2].rearrange("(p t) -> p t", p=P)
            )
            bg1_col = sb.tile([P, KT], FP, tag="bg1_col")
            nc.scalar.dma_start(
                bg1_col[:], vecs.ap()[3].rearrange("(p t) -> p t", p=P)
            )
            bg2_col = sb.tile([P, KT], FP, tag="bg2_col")
            nc.scalar.dma_start(
                bg2_col[:], vecs.ap()[4].rearrange("(p t) -> p t", p=P)
            )
            w1_t = load_nat_pairs("w1", w1, nc.scalar)

            # Warm the Scalar activation tables (exp, sqrt, sigmoid) so their
            # 1.3us ACT_TABLE_LOADs overlap the weight DMAs.
            warm = sb.tile([1, 1], FP, tag="warm")
            nc.vector.memset(warm[:], 1.0)
            warm2 = sb.tile([1, 1], FP, tag="warm2")
            nc.scalar.activation(warm2[:], warm[:], AF.Exp)
            nc.scalar.activation(warm2[:], warm[:], AF.Sqrt)
            nc.scalar.activation(warm2[:], warm[:], AF.Sigmoid)

            # ---- x0 in row form on partition 0 (fp32) ----
            rt = sb.tile([1, 1], FP, tag="rt")
            nc.vector.reciprocal(rt[:], vt[0:1, 5 * H : 5 * H + 1])
            ea = sb.tile([1, 1], FP, tag="ea")
            nc.scalar.activation(ea[:], rt[:], AF.Exp, scale=-1.0)  # exp(-1/tau)
            oma = sb.tile([1, 1], FP, tag="oma")
            nc.scalar.activation(oma[:], ea[:], AF.Copy, scale=-1.0, bias=1.0)
            nthr = sb.tile([1, 1], FP, tag="nthr")
            nc.scalar.activation(nthr[:], vt[0:1, 5 * H + 1 : 5 * H + 2], AF.Copy, scale=-1.0)

            xr = sb.tile([1, H], FP, tag="xr")
            nc.vector.tensor_add(xr[:], vt[0:1, 0:H], vt[0:1, H : 2 * H])
            nc.vector.tensor_scalar(
                xr[:], xr[:], oma[:], nthr[:], op0=ALU.mult, op1=ALU.add
            )
            nc.vector.tensor_scalar_max(xr[:], xr[:], 0.0)
            ssq = sb.tile([1, 1], FP, tag="ssq")
            sqj = jk.tile([1, H], FP, tag="sqj")
            nc.vector.scalar_tensor_tensor(
                sqj[:], xr[:], 1.0, xr[:], op0=ALU.mult, op1=ALU.mult,
                accum_out=ssq[:],
            )
            nrm = sb.tile([1, 1], FP, tag="nrm")
            nc.scalar.activation(nrm[:], ssq[:], AF.Sqrt)
            nc.scalar.activation(nrm[:], nrm[:], AF.Copy, bias=1e-8)
            inv = sb.tile([1, 1], FP, tag="inv")
            nc.vector.reciprocal(inv[:], nrm[:])
            nc.vector.tensor_scalar_mul(xr[:], xr[:], inv[:])
            xrb = sb.tile([1, H], BF, tag="xrb")
            nc.scalar.activation(xrb[:], xr[:], AF.Copy)

            xb = sb.tile([P, H], BF, tag="xb")
            nc.gpsimd.partition_broadcast(xb[:], xrb[0:1, :])

            # ---- fused row-dot matvec: acc[p, t] = sum_j W[t*128+p, j]*v[j]
            def matvec_nat(tiles, vb, acc):
                for t in range(KT):
                    w_ap = tiles[t // 4][:, t % 4, :]
                    junk = jk.tile([P, H], BF, tag="jv")
                    nc.vector.scalar_tensor_tensor(
                        junk[:], w_ap, 1.0, vb[:], op0=ALU.mult, op1=ALU.mult,
                        accum_out=acc[:, t : t + 1],
                    )

            # Stage A: out_c = relu(W1 @ x0 + bexc)
            outa = sb.tile([P, KT], FP, tag="outa")
            matvec_nat(w1_t, xb, outa[:])
            nc.vector.tensor_add(outa[:], outa[:], bexc_col[:])
            nc.vector.tensor_scalar_max(outa[:], outa[:], 0.0)
            outa_row = sb.tile([1, H], FP, tag="outa_row")
            nc.scalar.dma_start(outa_row[:], outa[:])
            outa_rowb = sb.tile([1, H], BF, tag="outa_rowb")
            nc.scalar.activation(outa_rowb[:], outa_row[:], AF.Copy)
            xb2 = sb.tile([P, H], BF, tag="xb2")
            nc.gpsimd.partition_broadcast(xb2[:], outa_rowb[0:1, :])

            # Stage B: hp = W2 @ out_c  (bg1 is added once after the gather)
            hp = sb.tile([P, KT], FP, tag="hp")
            matvec_nat(w2_t, xb2, hp[:])

            # ---- AllGather the 4KB partials (ncfw), triggered on hp ----
            cc_in = dram.tile([1, H], FP, tag="cc_in")
            cc_out = dram.tile([NCORES, H], FP, tag="cc_out")
            nc.scalar.dma_start(cc_in[:], hp[:])  # col->row, storage order
            nc.gpsimd.collective_compute(
                "AllGather",
                ALU.bypass,
                replica_groups=[list(range(NCORES))],
                ins=[cc_in[:]],
                outs=[cc_out[:]],
            )
            # pull each gathered row back as a column tile (contiguous 32B
            # per-partition chunks), split across both hwdge rings
            colt = []
            for k in range(NCORES):
                t = sb.tile([P, KT], FP, tag=f"colt{k}")
                eng = nc.scalar if k % 2 == 0 else nc.sync
                eng.dma_start(t[:], cc_out[k, :].rearrange("(p t) -> p t", p=P))
                colt.append(t)

            # r = relu(sum of all 8 partials + bg1), in column form
            s0 = sb.tile([P, KT], FP, tag="s0")
            nc.vector.tensor_add(s0[:], colt[0][:], colt[1][:])
            s1 = sb.tile([P, KT], FP, tag="s1")
            nc.vector.tensor_add(s1[:], colt[2][:], colt[3][:])
            s2 = sb.tile([P, KT], FP, tag="s2")
            nc.vector.tensor_add(s2[:], colt[4][:], colt[5][:])
            s3 = sb.tile([P, KT], FP, tag="s3")
            nc.vector.tensor_add(s3[:], colt[6][:], colt[7][:])
            nc.vector.tensor_add(s0[:], s0[:], s1[:])
            nc.vector.tensor_add(s2[:], s2[:], s3[:])
            nc.vector.tensor_add(s0[:], s0[:], s2[:])
            r_col = sb.tile([P, KT], FP, tag="r_col")
            nc.vector.tensor_add(r_col[:], s0[:], bg1_col[:])
            nc.vector.tensor_scalar_max(r_col[:], r_col[:], 0.0)

            # row-ize + bf16 + broadcast for the stage-C row-dots
            r_row = sb.tile([1, H], FP, tag="r_row")
            nc.scalar.dma_start(r_row[:], r_col[:])
            r_rowb = sb.tile([1, H], BF, tag="r_rowb")
            nc.scalar.activation(r_rowb[:], r_row[:], AF.Copy)
            xb3 = sb.tile([P, H], BF, tag="xb3")
            nc.gpsimd.partition_broadcast(xb3[:], r_rowb[0:1, :])

            # Stage C: s = sigmoid(-(W3 @ r + bg2)); final = out_c * s
            zb = sb.tile([P, KT], FP, tag="zb")
            matvec_nat(w3_t, xb3, zb[:])
            nc.vector.tensor_add(zb[:], zb[:], bg2_col[:])
            s_col = sb.tile([P, KT], FP, tag="s_col")
            nc.scalar.activation(s_col[:], zb[:], AF.Sigmoid, scale=-1.0)
            fin_t = sb.tile([P, KT], FP, tag="fin_t")
            nc.vector.tensor_mul(fin_t[:], outa[:], s_col[:])
            nc.sync.dma_start(fin.ap(), fin_t[:])

    nc.compile()
    return nc


def get_nc():
    if "nc" not in _CACHE:
        _CACHE["nc"] = _build_nc()
    return _CACHE["nc"]


def make_in_maps(inputs):
    """Slice the full inputs into 8 per-core input dicts (layout prep only).

    RHO is the p-major storage permutation: the device keeps the exchanged
    1024-vectors in storage order s with natural index rho[s] = (s%8)*128 +
    s//8, which makes every on-device transpose DMA contiguous.  The
    contractions are order-invariant, so we permute the matching weight
    columns / bias entries here and un-permute the final output on the host.
    """
    Wexc = np.asarray(inputs["Wexc"], dtype=np.float32)
    Wg1 = np.asarray(inputs["Wg1"], dtype=np.float32)
    Wg2 = np.asarray(inputs["Wg2"], dtype=np.float32)
    blat = np.asarray(inputs["blat_e"], dtype=np.float32)
    bfb = np.asarray(inputs["bfb_e"], dtype=np.float32)
    bexc = np.asarray(inputs["bexc"], dtype=np.float32)
    bg1 = np.asarray(inputs["bg1"], dtype=np.float32)
    bg2 = np.asarray(inputs["bg2"], dtype=np.float32)
    tau = np.asarray(inputs["tau_exc"], dtype=np.float32)
    thr = np.asarray(inputs["threshold"], dtype=np.float32)

    s_idx = np.arange(H)
    rho = (s_idx % KT) * P + s_idx // KT  # storage -> natural

    in_maps = []
    for c in range(NCORES):
        sl = slice(c * H, (c + 1) * H)
        srow = np.zeros((H,), np.float32)
        srow[0], srow[1] = tau[c], thr[c]
        vecs = np.stack(
            [blat[c, 0], bfb[c, 0], bexc[c, 0][rho], bg1[rho], bg2[sl][rho], srow]
        )
        in_maps.append(
            {
                "w1": np.ascontiguousarray(Wexc[c, 0]).astype(ml_dtypes.bfloat16),
                "w2": np.ascontiguousarray(Wg1[:, sl][:, rho]).astype(
                    ml_dtypes.bfloat16
                ),
                "w3": np.ascontiguousarray(Wg2[sl][:, rho]).astype(
                    ml_dtypes.bfloat16
                ),
                "vecs": np.ascontiguousarray(vecs),
            }
        )
    return in_maps


def kernel(**inputs):
    nc = get_nc()
    in_maps = make_in_maps(inputs)
    res = run_bass_kernel_spmd(nc, in_maps, core_ids=list(range(NCORES)))
    _CACHE["last_result"] = res
    chunks = []
    for c in range(NCORES):
        st = res.results[c]["final"]  # [P, KT], storage s = p*KT + t
        chunks.append(np.ascontiguousarray(st.T).reshape(-1))  # natural t*P+p
    return np.concatenate(chunks).astype(np.float32)


# revision 21
# speedup vs baseline: 1.0384x; 1.0262x over previous
"""Trainium2 Bass kernel for nn_CanonicalMicrocircuit (gnn_message_passing).

Math note: the reference module starts from all-zero recurrent state and only
returns `all_out * (1 - g)`, so every einsum against the zero state vanishes,
the inhibitory population and the inter-column lateral tensor are dead code,
and only layer 0 of the excitatory update survives:

    x0_c  = relu((1-exp(-1/tau_c)) * (blat_e[c,0] + bfb_e[c,0]) - thr_c)
    x0_c /= (||x0_c|| + 1e-8)
    out_c = relu(Wexc[c,0] @ x0_c + bexc[c,0])            # [H] per column
    h     = sum_c Wg1[:, cH:(c+1)H] @ out_c + bg1         # [H]
    r     = relu(h)
    g_c   = sigmoid(Wg2[cH:(c+1)H, :] @ r + bg2[cH:(c+1)H])
    final_c = out_c * (1 - g_c)                           # concat -> [C*H]

Sharding: one column per NeuronCore (C == 8 == n_cores).  Each core holds its
column's Wexc slice plus the matching column-block of Wg1 and row-block of
Wg2.  The only communication is one 4 KB AllGather of the per-core Wg1
partial products, summed locally on every core.

v6 (vs the 112 us baseline):
  * Weights travel as bf16 (host-cast): 6.3 MB instead of 12.6 MB of HBM
    traffic.
  * All compute on DVE/Scalar/GpSimd (row-dot matvecs with accum_out against
    partition-broadcast vectors).  The PE is unused, which removes the
    baseline's ~15 us serialized tiny-matmul tail; the gathered partials are
    pulled back from the CC output as column tiles and summed with plain
    vector adds instead of PE transposes.
  * w1 streams on the ACT hwdge ring (behind the small vecs loads) while
    w2/w3 stream on the SP ring, so stage A's weights are not queued behind
    stage B/C's.
  * Activation tables (exp/sqrt/sigmoid) are pre-warmed so their 1.3 us
    ACT_TABLE_LOADs stay off the critical path.
"""

import numpy as np
import ml_dtypes

import concourse.bass as bass
import concourse.bacc as bacc
import concourse.mybir as mybir
import concourse.tile as tile
from concourse.bass_utils import run_bass_kernel_spmd

C = 8
F = 512
L = 4
H = 1024
NCORES = 8
P = 128
KT = H // P  # 8 row/k tiles per 1024 dim
FP = mybir.dt.float32
BF = mybir.dt.bfloat16

_CACHE = {}


def _build_nc():
    nc = bacc.Bacc(
        "TRN2",
        target_bir_lowering=False,
        debug=False,
        enable_asserts=False,
        num_devices=NCORES,
    )

    w1 = nc.dram_tensor("w1", [H, H], BF, kind="ExternalInput")  # Wexc[c,0] natural
    w2 = nc.dram_tensor("w2", [H, H], BF, kind="ExternalInput")  # Wg1[:,blk][:,rho]
    w3 = nc.dram_tensor("w3", [H, H], BF, kind="ExternalInput")  # Wg2[blk][:,rho]
    vecs = nc.dram_tensor("vecs", [6, H], FP, kind="ExternalInput")
    # rows (rho = p-major storage permutation, see make_in_maps):
    # 0=blat, 1=bfb, 2=bexc[rho], 3=bg1[rho], 4=bg2[rho], 5=[tau, thr]
    fin = nc.dram_tensor("final", [P, KT], FP, kind="ExternalOutput")

    AF = mybir.ActivationFunctionType
    ALU = mybir.AluOpType

    with tile.TileContext(nc) as tc:
        with (
            tc.tile_pool(name="sb", bufs=1) as sb,
            tc.tile_pool(name="jk", bufs=2) as jk,
            tc.tile_pool(name="dram", bufs=1, space="DRAM") as dram,
        ):
            # ---- stage B/C weights on the SP hwdge ring ----
            def load_nat_pairs(name, dram_t, engine):
                tiles = []
                for a in range(KT // 4):
                    t = sb.tile([P, 4, H], BF, tag=f"{name}{a}")
                    src = dram_t.ap()[4 * a * P : 4 * (a + 1) * P, :].rearrange(
                        "(t p) i -> p t i", p=P
                    )
                    engine.dma_start(t[:], src)
                    tiles.append(t)
                return tiles  # tiles[a][:, b, :] is row-tile 4a+b

            # stage-A weights split across rings: half leads the SP ring so
            # stage A is never queued behind all of stage B/C's traffic
            def load_w1_half(a, eng):
                t = sb.tile([P, 4, H], BF, tag=f"w1{a}")
                src = w1.ap()[4 * a * P : 4 * (a + 1) * P, :].rearrange(
                    "(t p) i -> p t i", p=P
                )
                eng.dma_start(t[:], src)
                return t

            w1_t = [load_w1_half(0, nc.sync)]
            w2_t = load_nat_pairs("w2", w2, nc.sync)
            w3_t = load_nat_pairs("w3", w3, nc.sync)

            # ---- small loads on the ACT hwdge ring ----
            vt = sb.tile([1, 6 * H], FP, tag="vecs")
            nc.scalar.dma_start(
                vt[:], vecs.ap().rearrange("a b -> (a b)").rearrange("(x n) -> x n", x=1)
            )
            bexc_col = sb.tile([P, KT], FP, tag="bexc_col")
            nc.scalar.dma_start(
                bexc_col[:], vecs.ap()[2].rearrange("(p t) -> p t", p=P)
            )
            bg1_col = sb.tile([P, KT], FP, tag="bg1_col")
            nc.scalar.dma_start(
                bg1_col[:], vecs.ap()[3].rearrange("(p t) -> p t", p=P)
            )
            bg2_col = sb.tile([P, KT], FP, tag="bg2_col")
            nc.scalar.dma_start(
                bg2_col[:], vecs.ap()[4].rearrange("(p t) -> p t", p=P)
            )
            w1_t.append(load_w1_half(1, nc.scalar))

            # Warm the Scalar activation tables (exp, sqrt, sigmoid) so their
            # 1.3us ACT_TABLE_LOADs overlap the weight DMAs.
            warm = sb.tile([1, 1], FP, tag="warm")
            nc.vector.memset(warm[:], 1.0)
            warm2 = sb.tile([1, 1], FP, tag="warm2")
            nc.scalar.activation(warm2[:], warm[:], AF.Exp)
            nc.scalar.activation(warm2[:], warm[:], AF.Sqrt)
            nc.scalar.activation(warm2[:], warm[:], AF.Sigmoid)

            # ---- x0 in row form on partition 0 (fp32) ----
            rt = sb.tile([1, 1], FP, tag="rt")
            nc.vector.reciprocal(rt[:], vt[0:1, 5 * H : 5 * H + 1])
            ea = sb.tile([1, 1], FP, tag="ea")
            nc.scalar.activation(ea[:], rt[:], AF.Exp, scale=-1.0)  # exp(-1/tau)
            oma = sb.tile([1, 1], FP, tag="oma")
            nc.scalar.activation(oma[:], ea[:], AF.Copy, scale=-1.0, bias=1.0)
            nthr = sb.tile([1, 1], FP, tag="nthr")
            nc.scalar.activation(nthr[:], vt[0:1, 5 * H + 1 : 5 * H + 2], AF.Copy, scale=-1.0)

            # Un-normalized x0 is broadcast immediately; by linearity
            # W1 @ (x/||x||) = (W1 @ x) / ||x||, so the norm reciprocal is
            # folded into stage A's epilogue while ||x|| computes in parallel.
            xr = sb.tile([1, H], FP, tag="xr")
            nc.vector.tensor_add(xr[:], vt[0:1, 0:H], vt[0:1, H : 2 * H])
            nc.vector.tensor_scalar(
                xr[:], xr[:], oma[:], nthr[:], op0=ALU.mult, op1=ALU.add
            )
            nc.vector.tensor_scalar_max(xr[:], xr[:], 0.0)
            xrb = sb.tile([1, H], BF, tag="xrb")
            nc.scalar.activation(xrb[:], xr[:], AF.Copy)
            xb = sb.tile([P, H], BF, tag="xb")
            nc.gpsimd.partition_broadcast(xb[:], xrb[0:1, :])

            ssq = sb.tile([1, 1], FP, tag="ssq")
            sqj = jk.tile([1, H], FP, tag="sqj")
            nc.vector.scalar_tensor_tensor(
                sqj[:], xr[:], 1.0, xr[:], op0=ALU.mult, op1=ALU.mult,
                accum_out=ssq[:],
            )
            nrm = sb.tile([1, 1], FP, tag="nrm")
            nc.scalar.activation(nrm[:], ssq[:], AF.Sqrt)
            nc.scalar.activation(nrm[:], nrm[:], AF.Copy, bias=1e-8)
            inv = sb.tile([1, 1], FP, tag="inv")
            nc.vector.reciprocal(inv[:], nrm[:])
            invb = sb.tile([P, 1], FP, tag="invb")
            nc.gpsimd.partition_broadcast(invb[:], inv[0:1, :])

            # ---- fused row-dot matvec: acc[p, t] = sum_j W[t*128+p, j]*v[j]
            def matvec_nat(tiles, vb, acc):
                for t in range(KT):
                    w_ap = tiles[t // 4][:, t % 4, :]
                    junk = jk.tile([P, H], BF, tag="jv")
                    nc.vector.scalar_tensor_tensor(
                        junk[:], w_ap, 1.0, vb[:], op0=ALU.mult, op1=ALU.mult,
                        accum_out=acc[:, t : t + 1],
                    )

            # Stage A: out_c = relu((W1 @ x)/||x|| + bexc)
            outa = sb.tile([P, KT], FP, tag="outa")
            matvec_nat(w1_t, xb, outa[:])
            nc.vector.tensor_scalar_mul(outa[:], outa[:], invb[:])
            nc.vector.tensor_add(outa[:], outa[:], bexc_col[:])
            nc.vector.tensor_scalar_max(outa[:], outa[:], 0.0)
            outa_row = sb.tile([1, H], FP, tag="outa_row")
            nc.scalar.dma_start(outa_row[:], outa[:])
            outa_rowb = sb.tile([1, H], BF, tag="outa_rowb")
            nc.scalar.activation(outa_rowb[:], outa_row[:], AF.Copy)
            xb2 = sb.tile([P, H], BF, tag="xb2")
            nc.gpsimd.partition_broadcast(xb2[:], outa_rowb[0:1, :])

            # Stage B: hp = W2 @ out_c  (bg1 is added once after the gather)
            hp = sb.tile([P, KT], FP, tag="hp")
            matvec_nat(w2_t, xb2, hp[:])

            # ---- AllGather the 4KB partials (ncfw), triggered on hp ----
            cc_in = dram.tile([1, H], FP, tag="cc_in")
            cc_out = dram.tile([NCORES, H], FP, tag="cc_out")
            nc.scalar.dma_start(cc_in[:], hp[:])  # col->row, storage order
            nc.gpsimd.collective_compute(
                "AllGather",
                ALU.bypass,
                replica_groups=[list(range(NCORES))],
                ins=[cc_in[:]],
                outs=[cc_out[:]],
            )
            # pull each gathered row back as a column tile (contiguous 32B
            # per-partition chunks), split across both hwdge rings
            colt = []
            for k in range(NCORES):
                t = sb.tile([P, KT], FP, tag=f"colt{k}")
                eng = nc.scalar if k % 2 == 0 else nc.sync
                eng.dma_start(t[:], cc_out[k, :].rearrange("(p t) -> p t", p=P))
                colt.append(t)

            # r = relu(sum of all 8 partials + bg1), in column form
            s0 = sb.tile([P, KT], FP, tag="s0")
            nc.vector.tensor_add(s0[:], colt[0][:], colt[1][:])
            s1 = sb.tile([P, KT], FP, tag="s1")
            nc.vector.tensor_add(s1[:], colt[2][:], colt[3][:])
            s2 = sb.tile([P, KT], FP, tag="s2")
            nc.vector.tensor_add(s2[:], colt[4][:], colt[5][:])
            s3 = sb.tile([P, KT], FP, tag="s3")
            nc.vector.tensor_add(s3[:], colt[6][:], colt[7][:])
            nc.vector.tensor_add(s0[:], s0[:], s1[:])
            nc.vector.tensor_add(s2[:], s2[:], s3[:])
            nc.vector.tensor_add(s0[:], s0[:], s2[:])
            r_col = sb.tile([P, KT], FP, tag="r_col")
            nc.vector.tensor_add(r_col[:], s0[:], bg1_col[:])
            nc.vector.tensor_scalar_max(r_col[:], r_col[:], 0.0)

            # row-ize + bf16 + broadcast for the stage-C row-dots
            r_row = sb.tile([1, H], FP, tag="r_row")
            nc.scalar.dma_start(r_row[:], r_col[:])
            r_rowb = sb.tile([1, H], BF, tag="r_rowb")
            nc.scalar.activation(r_rowb[:], r_row[:], AF.Copy)
            xb3 = sb.tile([P, H], BF, tag="xb3")
            nc.gpsimd.partition_broadcast(xb3[:], r_rowb[0:1, :])

            # Stage C: s = sigmoid(-(W3 @ r + bg2)); final = out_c * s
            zb = sb.tile([P, KT], FP, tag="zb")
            matvec_nat(w3_t, xb3, zb[:])
            nc.vector.tensor_add(zb[:], zb[:], bg2_col[:])
            s_col = sb.tile([P, KT], FP, tag="s_col")
            nc.scalar.activation(s_col[:], zb[:], AF.Sigmoid, scale=-1.0)
            fin_t = sb.tile([P, KT], FP, tag="fin_t")
            nc.vector.tensor_mul(fin_t[:], outa[:], s_col[:])
            nc.sync.dma_start(fin.ap(), fin_t[:])

    nc.compile()
    return nc


def get_nc():
    if "nc" not in _CACHE:
        _CACHE["nc"] = _build_nc()
    return _CACHE["nc"]


def make_in_maps(inputs):
    """Slice the full inputs into 8 per-core input dicts (layout prep only).

    RHO is the p-major storage permutation: the device keeps the exchanged
    1024-vectors in storage order s with natural index rho[s] = (s%8)*128 +
    s//8, which makes every on-device transpose DMA contiguous.  The
    contractions are order-invariant, so we permute the matching weight
    columns / bias entries here and un-permute the final output on the host.
    """
    Wexc = np.asarray(inputs["Wexc"], dtype=np.float32)
    Wg1 = np.asarray(inputs["Wg1"], dtype=np.float32)
    Wg2 = np.asarray(inputs["Wg2"], dtype=np.float32)
    blat = np.asarray(inputs["blat_e"], dtype=np.float32)
    bfb = np.asarray(inputs["bfb_e"], dtype=np.float32)
    bexc = np.asarray(inputs["bexc"], dtype=np.float32)
    bg1 = np.asarray(inputs["bg1"], dtype=np.float32)
    bg2 = np.asarray(inputs["bg2"], dtype=np.float32)
    tau = np.asarray(inputs["tau_exc"], dtype=np.float32)
    thr = np.asarray(inputs["threshold"], dtype=np.float32)

    s_idx = np.arange(H)
    rho = (s_idx % KT) * P + s_idx // KT  # storage -> natural

    in_maps = []
    for c in range(NCORES):
        sl = slice(c * H, (c + 1) * H)
        srow = np.zeros((H,), np.float32)
        srow[0], srow[1] = tau[c], thr[c]
        vecs = np.stack(
            [blat[c, 0], bfb[c, 0], bexc[c, 0][rho], bg1[rho], bg2[sl][rho], srow]
        )
        in_maps.append(
            {
                "w1": np.ascontiguousarray(Wexc[c, 0]).astype(ml_dtypes.bfloat16),
                "w2": np.ascontiguousarray(Wg1[:, sl][:, rho]).astype(
                    ml_dtypes.bfloat16
                ),
                "w3": np.ascontiguousarray(Wg2[sl][:, rho]).astype(
                    ml_dtypes.bfloat16
                ),
                "vecs": np.ascontiguousarray(vecs),
            }
        )
    return in_maps


def kernel(**inputs):
    nc = get_nc()
    in_maps = make_in_maps(inputs)
    res = run_bass_kernel_spmd(nc, in_maps, core_ids=list(range(NCORES)))
    _CACHE["last_result"] = res
    chunks = []
    for c in range(NCORES):
        st = res.results[c]["final"]  # [P, KT], storage s = p*KT + t
        chunks.append(np.ascontiguousarray(st.T).reshape(-1))  # natural t*P+p
    return np.concatenate(chunks).astype(np.float32)


# revision 28
# speedup vs baseline: 1.0769x; 1.0371x over previous
"""Trainium2 Bass kernel for nn_CanonicalMicrocircuit (gnn_message_passing).

Math note: the reference module starts from all-zero recurrent state and only
returns `all_out * (1 - g)`, so every einsum against the zero state vanishes,
the inhibitory population and the inter-column lateral tensor are dead code,
and only layer 0 of the excitatory update survives:

    x0_c  = relu((1-exp(-1/tau_c)) * (blat_e[c,0] + bfb_e[c,0]) - thr_c)
    x0_c /= (||x0_c|| + 1e-8)
    out_c = relu(Wexc[c,0] @ x0_c + bexc[c,0])            # [H] per column
    h     = sum_c Wg1[:, cH:(c+1)H] @ out_c + bg1         # [H]
    r     = relu(h)
    g_c   = sigmoid(Wg2[cH:(c+1)H, :] @ r + bg2[cH:(c+1)H])
    final_c = out_c * (1 - g_c)                           # concat -> [C*H]

Sharding: one column per NeuronCore (C == 8 == n_cores).  Each core holds its
column's Wexc slice plus the matching column-block of Wg1 and row-block of
Wg2.  The only communication is one 4 KB AllGather of the per-core Wg1
partial products, summed locally on every core.

v6 (vs the 112 us baseline):
  * Weights travel as bf16 (host-cast): 6.3 MB instead of 12.6 MB of HBM
    traffic.
  * All compute on DVE/Scalar/GpSimd (row-dot matvecs with accum_out against
    partition-broadcast vectors).  The PE is unused, which removes the
    baseline's ~15 us serialized tiny-matmul tail; the gathered partials are
    pulled back from the CC output as column tiles and summed with plain
    vector adds instead of PE transposes.
  * w1 streams on the ACT hwdge ring (behind the small vecs loads) while
    w2/w3 stream on the SP ring, so stage A's weights are not queued behind
    stage B/C's.
  * Activation tables (exp/sqrt/sigmoid) are pre-warmed so their 1.3 us
    ACT_TABLE_LOADs stay off the critical path.
"""

import numpy as np
import ml_dtypes

import concourse.bass as bass
import concourse.bacc as bacc
import concourse.mybir as mybir
import concourse.tile as tile
from concourse.bass_utils import run_bass_kernel_spmd

C = 8
F = 512
L = 4
H = 1024
NCORES = 8
P = 128
KT = H // P  # 8 row/k tiles per 1024 dim
FP = mybir.dt.float32
BF = mybir.dt.bfloat16

_CACHE = {}


def _build_nc():
    nc = bacc.Bacc(
        "TRN2",
        target_bir_lowering=False,
        debug=False,
        enable_asserts=False,
        num_devices=NCORES,
    )

    w1 = nc.dram_tensor("w1", [H, H], BF, kind="ExternalInput")  # Wexc[c,0] natural
    w2 = nc.dram_tensor("w2", [H, H], BF, kind="ExternalInput")  # Wg1[:,blk][:,rho]
    # w3t[j, n] = Wg2[blk][rho[n], rho[j]] — PE rhs tiles, contraction rows j
    w3 = nc.dram_tensor("w3", [H, H], BF, kind="ExternalInput")
    vecs = nc.dram_tensor("vecs", [6, H], FP, kind="ExternalInput")
    # rows (rho = p-major storage permutation, see make_in_maps):
    # 0=blat, 1=bfb, 2=bexc[rho], 3=bg1[rho], 4=bg2[rho], 5=[tau, thr]
    fin = nc.dram_tensor("final", [1, H], FP, kind="ExternalOutput")

    AF = mybir.ActivationFunctionType
    ALU = mybir.AluOpType

    with tile.TileContext(nc) as tc:
        with (
            tc.tile_pool(name="sb", bufs=1) as sb,
            tc.tile_pool(name="jk", bufs=2) as jk,
            tc.tile_pool(name="ps_row", bufs=2, space="PSUM") as ps_row,
            tc.tile_pool(name="ps_tp", bufs=1, space="PSUM") as ps_tp,
            tc.tile_pool(name="dram", bufs=1, space="DRAM") as dram,
        ):
            # ---- stage B/C weights on the SP hwdge ring ----
            def load_nat_pairs(name, dram_t, engine):
                tiles = []
                for a in range(KT // 4):
                    t = sb.tile([P, 4, H], BF, tag=f"{name}{a}")
                    src = dram_t.ap()[4 * a * P : 4 * (a + 1) * P, :].rearrange(
                        "(t p) i -> p t i", p=P
                    )
                    engine.dma_start(t[:], src)
                    tiles.append(t)
                return tiles  # tiles[a][:, b, :] is row-tile 4a+b

            # stage-A weights split across rings: half leads the SP ring so
            # stage A is never queued behind all of stage B/C's traffic
            def load_w1_half(a, eng):
                t = sb.tile([P, 4, H], BF, tag=f"w1{a}")
                src = w1.ap()[4 * a * P : 4 * (a + 1) * P, :].rearrange(
                    "(t p) i -> p t i", p=P
                )
                eng.dma_start(t[:], src)
                return t

            w1_t = [load_w1_half(0, nc.sync)]
            w2_t = load_nat_pairs("w2", w2, nc.sync)
            w3_t = load_nat_pairs("w3", w3, nc.sync)

            # ---- small loads on the ACT hwdge ring ----
            vt = sb.tile([1, 6 * H], FP, tag="vecs")
            nc.scalar.dma_start(
                vt[:], vecs.ap().rearrange("a b -> (a b)").rearrange("(x n) -> x n", x=1)
            )
            bexc_col = sb.tile([P, KT], FP, tag="bexc_col")
            nc.scalar.dma_start(
                bexc_col[:], vecs.ap()[2].rearrange("(p t) -> p t", p=P)
            )
            bg1_col = sb.tile([P, KT], FP, tag="bg1_col")
            nc.scalar.dma_start(
                bg1_col[:], vecs.ap()[3].rearrange("(p t) -> p t", p=P)
            )
            w1_t.append(load_w1_half(1, nc.scalar))

            # PE constants
            ones_8 = sb.tile([NCORES, 1], FP, tag="ones_8")
            one_11 = sb.tile([1, 1], FP, tag="one_11")
            nc.vector.memset(ones_8[:], 1.0)
            nc.vector.memset(one_11[:], 1.0)

            # Warm the Scalar activation tables (exp, sqrt, sigmoid) so their
            # 1.3us ACT_TABLE_LOADs overlap the weight DMAs.
            warm = sb.tile([1, 1], FP, tag="warm")
            nc.vector.memset(warm[:], 1.0)
            warm2 = sb.tile([1, 1], FP, tag="warm2")
            nc.scalar.activation(warm2[:], warm[:], AF.Exp)
            nc.scalar.activation(warm2[:], warm[:], AF.Sqrt)
            nc.scalar.activation(warm2[:], warm[:], AF.Sigmoid)

            # ---- x0 in row form on partition 0 (fp32) ----
            rt = sb.tile([1, 1], FP, tag="rt")
            nc.vector.reciprocal(rt[:], vt[0:1, 5 * H : 5 * H + 1])
            ea = sb.tile([1, 1], FP, tag="ea")
            nc.scalar.activation(ea[:], rt[:], AF.Exp, scale=-1.0)  # exp(-1/tau)
            oma = sb.tile([1, 1], FP, tag="oma")
            nc.scalar.activation(oma[:], ea[:], AF.Copy, scale=-1.0, bias=1.0)
            nthr = sb.tile([1, 1], FP, tag="nthr")
            nc.scalar.activation(nthr[:], vt[0:1, 5 * H + 1 : 5 * H + 2], AF.Copy, scale=-1.0)

            xr = sb.tile([1, H], FP, tag="xr")
            nc.vector.tensor_add(xr[:], vt[0:1, 0:H], vt[0:1, H : 2 * H])
            nc.vector.tensor_scalar(
                xr[:], xr[:], oma[:], nthr[:], op0=ALU.mult, op1=ALU.add
            )
            nc.vector.tensor_scalar_max(xr[:], xr[:], 0.0)
            ssq = sb.tile([1, 1], FP, tag="ssq")
            sqj = jk.tile([1, H], FP, tag="sqj")
            nc.vector.scalar_tensor_tensor(
                sqj[:], xr[:], 1.0, xr[:], op0=ALU.mult, op1=ALU.mult,
                accum_out=ssq[:],
            )
            nrm = sb.tile([1, 1], FP, tag="nrm")
            nc.scalar.activation(nrm[:], ssq[:], AF.Sqrt)
            nc.scalar.activation(nrm[:], nrm[:], AF.Copy, bias=1e-8)
            inv = sb.tile([1, 1], FP, tag="inv")
            nc.vector.reciprocal(inv[:], nrm[:])
            nc.vector.tensor_scalar_mul(xr[:], xr[:], inv[:])
            xrb = sb.tile([1, H], BF, tag="xrb")
            nc.scalar.activation(xrb[:], xr[:], AF.Copy)

            xb = sb.tile([P, H], BF, tag="xb")
            nc.gpsimd.partition_broadcast(xb[:], xrb[0:1, :])

            # ---- fused row-dot matvec: acc[p, t] = sum_j W[t*128+p, j]*v[j]
            def matvec_nat(tiles, vb, acc):
                for t in range(KT):
                    w_ap = tiles[t // 4][:, t % 4, :]
                    junk = jk.tile([P, H], BF, tag="jv")
                    nc.vector.scalar_tensor_tensor(
                        junk[:], w_ap, 1.0, vb[:], op0=ALU.mult, op1=ALU.mult,
                        accum_out=acc[:, t : t + 1],
                    )

            # Stage A: out_c = relu(W1 @ x0 + bexc)
            outa = sb.tile([P, KT], FP, tag="outa")
            matvec_nat(w1_t, xb, outa[:])
            nc.vector.tensor_add(outa[:], outa[:], bexc_col[:])
            nc.vector.tensor_scalar_max(outa[:], outa[:], 0.0)
            outa_row = sb.tile([1, H], FP, tag="outa_row")
            nc.scalar.dma_start(outa_row[:], outa[:])
            outa_rowb = sb.tile([1, H], BF, tag="outa_rowb")
            nc.scalar.activation(outa_rowb[:], outa_row[:], AF.Copy)
            xb2 = sb.tile([P, H], BF, tag="xb2")
            nc.gpsimd.partition_broadcast(xb2[:], outa_rowb[0:1, :])

            # Stage B: hp = W2 @ out_c + bg1/8 (so the gathered sum has bg1)
            hp = sb.tile([P, KT], FP, tag="hp")
            matvec_nat(w2_t, xb2, hp[:])
            nc.vector.scalar_tensor_tensor(
                hp[:], bg1_col[:], 0.125, hp[:], op0=ALU.mult, op1=ALU.add
            )

            # ---- AllGather the 4KB partials (ncfw), triggered on hp ----
            cc_in = dram.tile([1, H], FP, tag="cc_in")
            cc_out = dram.tile([NCORES, H], FP, tag="cc_out")
            nc.scalar.dma_start(cc_in[:], hp[:])  # col->row, storage order
            nc.gpsimd.collective_compute(
                "AllGather",
                ALU.bypass,
                replica_groups=[list(range(NCORES))],
                ins=[cc_in[:]],
                outs=[cc_out[:]],
            )
            # gathered rows as two [8, 512] tiles; PE sums them by a ones rhs
            # into transposed-column form: psRc[p, t] = r_storage[t*128+p]
            agt_a = sb.tile([NCORES, H // 2], FP, tag="agt_a")
            nc.scalar.dma_start(agt_a[:], cc_out[:, 0 : H // 2])
            agt_b = sb.tile([NCORES, H // 2], FP, tag="agt_b")
            nc.sync.dma_start(agt_b[:], cc_out[:, H // 2 : H])
            psRc = ps_tp.tile([P, KT], FP, tag="tp")
            for t in range(KT):
                half = agt_a if t < 4 else agt_b
                nc.tensor.matmul(
                    psRc[:, t : t + 1],
                    half[:, (t % 4) * P : (t % 4 + 1) * P],
                    ones_8[:],
                    start=True,
                    stop=True,
                )
            rT_b = sb.tile([P, KT], BF, tag="rT_b")
            nc.scalar.activation(rT_b[:], psRc[:], AF.Relu)

            # Stage C on the PE: psum[0, n] = sum_k rT[:,k]^T @ w3t[k-tile][:, n]
            # (+ bg2 via a ones-row matmul); s = sigmoid(-psum) in storage row
            # order, matching outa_row.
            s_row = sb.tile([1, H], FP, tag="s_row")
            for h in range(2):
                ps = ps_row.tile([1, H // 2], FP, tag="row")
                sl = slice(h * (H // 2), (h + 1) * (H // 2))
                for k in range(KT):
                    nc.tensor.matmul(
                        ps[:],
                        rT_b[:, k : k + 1],
                        w3_t[k // 4][:, k % 4, sl],
                        start=(k == 0),
                        stop=False,
                    )
                nc.tensor.matmul(
                    ps[:],
                    one_11[:],
                    vt[0:1, 4 * H + h * (H // 2) : 4 * H + (h + 1) * (H // 2)],
                    start=False,
                    stop=True,
                )
                nc.scalar.activation(s_row[0:1, sl], ps[:], AF.Sigmoid, scale=-1.0)

            # final = out_c * s  (storage-ordered row; host un-permutes)
            fin_row = sb.tile([1, H], FP, tag="fin_row")
            nc.vector.tensor_mul(fin_row[:], outa_row[:], s_row[:])
            nc.sync.dma_start(fin.ap(), fin_row[:])

    nc.compile()
    return nc


def get_nc():
    if "nc" not in _CACHE:
        _CACHE["nc"] = _build_nc()
    return _CACHE["nc"]


def make_in_maps(inputs):
    """Slice the full inputs into 8 per-core input dicts (layout prep only).

    RHO is the p-major storage permutation: the device keeps the exchanged
    1024-vectors in storage order s with natural index rho[s] = (s%8)*128 +
    s//8, which makes every on-device transpose DMA contiguous.  The
    contractions are order-invariant, so we permute the matching weight
    columns / bias entries here and un-permute the final output on the host.
    """
    Wexc = np.asarray(inputs["Wexc"], dtype=np.float32)
    Wg1 = np.asarray(inputs["Wg1"], dtype=np.float32)
    Wg2 = np.asarray(inputs["Wg2"], dtype=np.float32)
    blat = np.asarray(inputs["blat_e"], dtype=np.float32)
    bfb = np.asarray(inputs["bfb_e"], dtype=np.float32)
    bexc = np.asarray(inputs["bexc"], dtype=np.float32)
    bg1 = np.asarray(inputs["bg1"], dtype=np.float32)
    bg2 = np.asarray(inputs["bg2"], dtype=np.float32)
    tau = np.asarray(inputs["tau_exc"], dtype=np.float32)
    thr = np.asarray(inputs["threshold"], dtype=np.float32)

    s_idx = np.arange(H)
    rho = (s_idx % KT) * P + s_idx // KT  # storage -> natural

    in_maps = []
    for c in range(NCORES):
        sl = slice(c * H, (c + 1) * H)
        srow = np.zeros((H,), np.float32)
        srow[0], srow[1] = tau[c], thr[c]
        vecs = np.stack(
            [blat[c, 0], bfb[c, 0], bexc[c, 0][rho], bg1[rho], bg2[sl][rho], srow]
        )
        w3pp = Wg2[sl][np.ix_(rho, rho)]
        in_maps.append(
            {
                "w1": np.ascontiguousarray(Wexc[c, 0]).astype(ml_dtypes.bfloat16),
                "w2": np.ascontiguousarray(Wg1[:, sl][:, rho]).astype(
                    ml_dtypes.bfloat16
                ),
                "w3": np.ascontiguousarray(w3pp.T).astype(ml_dtypes.bfloat16),
                "vecs": np.ascontiguousarray(vecs),
            }
        )
    return in_maps


def kernel(**inputs):
    nc = get_nc()
    in_maps = make_in_maps(inputs)
    res = run_bass_kernel_spmd(nc, in_maps, core_ids=list(range(NCORES)))
    _CACHE["last_result"] = res
    chunks = []
    for c in range(NCORES):
        st = res.results[c]["final"].reshape(P, KT)  # storage s = p*KT + t
        chunks.append(np.ascontiguousarray(st.T).reshape(-1))  # natural t*P+p
    return np.concatenate(chunks).astype(np.float32)
